# revision 1
# baseline (speedup 1.0000x reference)
"""BitNetV3Attention (B=2, S=2048, H=16, DH=128, D=2048) on 8 TRN2 NeuronCores.

Strategy (tensor-parallel over heads + row-parallel o_proj):
  - Each core owns 2 of 16 heads. It computes Q^T/K^T (head-transposed,
    [DH, B*S]) and V ([B*S, DH]) for its heads from the full hidden states
    (replicated read), runs causal flash-style attention per (head, batch),
    producing normalized attn_out^T slices [256, B*S].
  - Two AllToAll collectives (one per local head slot) redistribute attn_out
    from head-sharded to sequence-sharded: core j ends with
    attn_out^T[:, rows_j] for ALL 2048 model dims, where rows_j are 512 rows
    of the [4096, 2048] token matrix. The first A2A fires after local head 0
    finishes and overlaps head 1's attention.
  - Each core computes its 512 output rows against the full Wo (no
    all-reduce needed; outputs concatenate on host). o_proj accumulates
    even d-tiles (from A2A#0) before odd ones (A2A#1) so it can start
    before the second collective lands. Wo slabs prefetch during attention.

All matmuls run in float32r (full-rate fp32 path, ~1e-3 relative error).
Softmax skips max-subtraction (scores are O(5), exp is safe in fp32); the
padding mask rides the ScalarE activation's per-partition bias; the causal
mask is a single [128, 1024] additive template sliced per diagonal tile.
"""
import sys
for _p in ('/opt/trn_rl_repo', '/root/.axon_site/_ro/trn_rl_repo'):
    if _p not in sys.path:
        sys.path.append(_p)

import numpy as np

import concourse.mybir as mybir
import concourse.tile as tile
from concourse import bacc, bass_utils

B, S, H, DH = 2, 2048, 16, 128
D = H * DH                  # 2048
NS = B * S                  # 4096
NC = 8                      # cores
HL = H // NC                # 2 local heads
DSL = HL * DH               # 256 (d-slice per core)
ROWS = NS // NC             # 512 output rows per core
SCALE = 1.0 / float(np.sqrt(DH))
F32 = mybir.dt.float32
F32R = mybir.dt.float32r
BF16 = mybir.dt.bfloat16
# dtype used for QKV/attention matmul operands (F32R or BF16) - A/B testable
MM_DT = F32R
EXP = mybir.ActivationFunctionType.Exp
NEG = -1.0e30

N_K = D // 128              # 16 contraction tiles
N_SC = NS // 512            # 8 s-chunks for QKV
N_QC = S // 512             # 4 q-chunks per batch


def build_bass(repeat=1, do_attn=True, do_a2a=True, do_oproj=True):
    nc = bacc.Bacc("TRN2", target_bir_lowering=False, debug=False, num_devices=NC)

    ht = nc.dram_tensor("ht", [D, NS], MM_DT, kind="ExternalInput").ap()
    wqt = nc.dram_tensor("wqt", [D, DSL], MM_DT, kind="ExternalInput").ap()
    wkt = nc.dram_tensor("wkt", [D, DSL], MM_DT, kind="ExternalInput").ap()
    wvt = nc.dram_tensor("wvt", [D, DSL], MM_DT, kind="ExternalInput").ap()
    wot = nc.dram_tensor("wot", [D, D], BF16, kind="ExternalInput").ap()
    pad = nc.dram_tensor("pad", [B, S], F32, kind="ExternalInput").ap()
    tri = nc.dram_tensor("tri", [128, 1024], F32, kind="ExternalInput").ap()
    onesd = nc.dram_tensor("ones", [128, 128], MM_DT, kind="ExternalInput").ap()
    idend = nc.dram_tensor("iden", [128, 128], MM_DT, kind="ExternalInput").ap()
    out = nc.dram_tensor("out", [ROWS, D], F32, kind="ExternalOutput").ap()

    with tile.TileContext(nc) as tc:
        with tc.tile_pool(name="dram", bufs=1, space="DRAM") as dram, \
             tc.tile_pool(name="const", bufs=1) as cpool:
            a2a_in = [dram.tile([NC, DH, 512], BF16, name=f"a2a_in{h}") for h in range(HL)]
            a2a_out = [dram.tile([NC, DH, 512], BF16, name=f"a2a_out{h}") for h in range(HL)]

            tri_sb = cpool.tile([128, 1024], F32)
            pad_sb = cpool.tile([128, B * 16], F32)
            ones_sb = cpool.tile([128, 128], MM_DT)
            iden_sb = cpool.tile([128, 128], MM_DT)
            nc.sync.dma_start(tri_sb[:], tri)
            nc.sync.dma_start(
                pad_sb[:].rearrange("p (b t) -> p b t", b=B),
                pad.rearrange("b (t p) -> p b t", p=128),
            )
            nc.sync.dma_start(ones_sb[:], onesd)
            nc.sync.dma_start(iden_sb[:], idend)

            for _rep in range(repeat):
                _emit_body(nc, tc, a2a_in, a2a_out, tri_sb, pad_sb, ones_sb,
                           iden_sb, ht, wqt, wkt, wvt, wot, out,
                           do_attn=do_attn, do_a2a=do_a2a, do_oproj=do_oproj)
    nc.compile()
    return nc


def _emit_qkv(nc, tc, qt_sb, kt_sb, v_sb, iden_sb, ht, wqt, wkt, wvt):
    with tc.tile_pool(name="wts", bufs=1) as wpool, \
         tc.tile_pool(name="hts", bufs=2) as hpool, \
         tc.tile_pool(name="vtt", bufs=2) as vpool, \
         tc.tile_pool(name="ps1", bufs=1, space="PSUM") as pp1:
        w_sb = {}
        for nm, src in (("q", wqt), ("k", wkt), ("v", wvt)):
            w = wpool.tile([128, N_K * DSL], MM_DT, name=f"w{nm}")
            # k=0 tile first so the first matmuls start ~1us in, then the rest
            nc.sync.dma_start(w[:, 0:DSL], src[0:128, :])
            nc.sync.dma_start(
                w[:, DSL:].rearrange("p (t m) -> p t m", t=N_K - 1),
                src[128:, :].rearrange("(t p) m -> p t m", p=128),
            )
            w_sb[nm] = w

        ht_r = ht.rearrange("(k p) s -> p k s", p=128)
        for sc in range(N_SC):
            psq = [pp1.tile([128, 512], F32, tag=f"pq{h}", name=f"pq{h}") for h in range(HL)]
            psk = [pp1.tile([128, 512], F32, tag=f"pk{h}", name=f"pk{h}") for h in range(HL)]
            psvt = [pp1.tile([128, 512], F32, tag=f"pvt{h}", name=f"pvt{h}") for h in range(HL)]
            slabs = []
            for half in range(2):
                slab = hpool.tile([128, 8 * 512], MM_DT, tag="ht", name="htslab")
                nc.sync.dma_start(
                    slab[:].rearrange("p (k s) -> p k s", k=8),
                    ht_r[:, 8*half:8*half+8, 512*sc:512*sc+512])
                slabs.append(slab)
            for k in range(N_K):
                htt = slabs[k // 8][:, 512*(k % 8):512*(k % 8)+512]
                fl = dict(start=(k == 0), stop=(k == N_K - 1))
                for h in range(HL):
                    nc.tensor.matmul(
                        psq[h][:], w_sb["q"][:, DSL*k+128*h:DSL*k+128*h+128],
                        htt, **fl)
                    nc.tensor.matmul(
                        psk[h][:], w_sb["k"][:, DSL*k+128*h:DSL*k+128*h+128],
                        htt, **fl)
                    nc.tensor.matmul(
                        psvt[h][:], w_sb["v"][:, DSL*k+128*h:DSL*k+128*h+128],
                        htt, **fl)
            # drain PSUM -> SBUF, split across DVE and ACT
            vtt = []
            for h in range(HL):
                nc.vector.tensor_copy(
                    qt_sb[h][:, 512*sc:512*sc+512], psq[h][:])
                nc.scalar.copy(
                    kt_sb[h][:, 512*sc:512*sc+512], psk[h][:])
                vt = vpool.tile([128, 512], MM_DT, tag=f"vtt{h}", name=f"vtt{h}")
                if h == 0:
                    nc.vector.tensor_copy(vt[:], psvt[h][:])
                else:
                    nc.scalar.copy(vt[:], psvt[h][:])
                vtt.append(vt)
            # PE-transpose V chunk to natural [s, dh] layout
            for h in range(HL):
                for m in range(4):
                    ptp = pp1.tile([128, 128], MM_DT, tag="ptp", name="ptp", bufs=2)
                    nc.tensor.transpose(
                        ptp[:], vtt[h][:, 128*m:128*m+128], iden_sb[:])
                    st = 4 * sc + m
                    if (h + m) % 2 == 0:
                        nc.vector.tensor_copy(
                            v_sb[h][:, 128*st:128*st+128], ptp[:])
                    else:
                        nc.scalar.copy(
                            v_sb[h][:, 128*st:128*st+128], ptp[:])


def _emit_attention(nc, tc, qt_sb, kt_sb, v_sb, tri_sb, pad_sb, ones_sb,
                    a2a_in, a2a_out, do_a2a):
    with tc.tile_pool(name="att", bufs=1) as apool, \
         tc.tile_pool(name="ps2", bufs=1, space="PSUM") as pp2:
        for h in range(HL):
            for b in range(B):
                for qc in range(N_QC):
                    q0 = 512 * qc
                    n_sk = 4 * qc + 4
                    po = pp2.tile([128, 512], F32, tag="po", bufs=2, name="po")
                    pd = pp2.tile([128, 512], F32, tag="pd", bufs=2, name="pd")
                    for t in range(n_sk):
                        # columns sq < o are fully causal-masked; skip them
                        o = max(0, 128 * t - q0)
                        ps = pp2.tile([128, 512], F32, tag="ps", bufs=4, name="ps")
                        nc.tensor.matmul(
                            ps[:, o:512],
                            kt_sb[h][:, S*b+128*t:S*b+128*t+128],
                            qt_sb[h][:, S*b+q0+o:S*b+q0+512],
                            start=True, stop=True)
                        if t >= 4 * qc:  # diagonal block
                            nc.vector.tensor_add(
                                ps[:, o:512], ps[:, o:512], tri_sb[:, 512:1024-o])
                        ex = apool.tile([128, 512], MM_DT, tag="ex", bufs=6, name="ex")
                        nc.scalar.activation(
                            ex[:, o:512], ps[:, o:512], EXP,
                            bias=pad_sb[:, 16*b+t:16*b+t+1], scale=SCALE)
                        fl = dict(start=(t == 0), stop=(t == n_sk - 1))
                        st = 16 * b + t
                        nc.tensor.matmul(
                            po[:, o:512], v_sb[h][:, 128*st:128*st+128],
                            ex[:, o:512], **fl)
                        nc.tensor.matmul(
                            pd[:, o:512], ones_sb[:], ex[:, o:512], **fl)
                    rec = apool.tile([128, 512], F32, tag="rec", bufs=2, name="rec")
                    nc.vector.reciprocal(rec[:], pd[:])
                    ao = apool.tile([128, 512], BF16, tag="ao", bufs=2, name="ao")
                    nc.vector.tensor_mul(ao[:], po[:], rec[:])
                    nc.sync.dma_start(a2a_in[h][4*b+qc, :, :], ao[:])
            # ---- AllToAll for this head-slot (overlaps next head's attn) ----
            if do_a2a:
                nc.gpsimd.collective_compute(
                    "AllToAll", mybir.AluOpType.bypass,
                    replica_groups=[list(range(NC))],
                    ins=[a2a_in[h].opt()], outs=[a2a_out[h].opt()])


def _emit_oproj(nc, tc, opool, wopool, obpool, a2a_out, wot, out):
    # global d-tile g lives at a2a_out[g % 2][g // 2]; evens first so the
    # accumulation can start before A2A#1 lands.
    korder = [g for g in range(N_K) if g % 2 == 0] + \
             [g for g in range(N_K) if g % 2 == 1]
    with tc.tile_pool(name="ps4", bufs=4, space="PSUM") as pp4:
        at_sb = [opool.tile([128, 8 * 512], BF16, name=f"at{half}")
                 for half in range(2)]
        for half in range(2):          # half 0 = even g (head slot 0)
            nc.sync.dma_start(
                at_sb[half][:].rearrange("p (j s) -> p j s", j=8),
                a2a_out[half].rearrange("j p s -> p j s"))
        # wot rows (t p) with t = global d-tile; split parity for slabs
        wot_r2 = wot.rearrange("(t2 two p) e -> p two t2 e", p=128, two=2)
        for ne in range(4):
            slabs = []
            for half in range(2):
                sl = wopool.tile([128, 8 * 512], BF16, tag=f"wo{half}",
                                 name=f"wo{half}", bufs=2)
                nc.sync.dma_start(
                    sl[:].rearrange("p (t e) -> p t e", t=8),
                    wot_r2[:, half, :, 512*ne:512*ne+512])
                slabs.append(sl)
            for m in range(4):
                pout = pp4.tile([128, 512], F32, tag="pout", name="pout", bufs=6)
                for i, g in enumerate(korder):
                    half, j = g % 2, g // 2
                    nc.tensor.matmul(
                        pout[:],
                        at_sb[half][:, 512*j+128*m:512*j+128*m+128],
                        slabs[half][:, 512*j:512*j+512],
                        start=(i == 0), stop=(i == N_K - 1))
                ob = obpool.tile([128, 512], F32, tag="ob", name="ob", bufs=4)
                if (ne + m) % 2 == 0:
                    nc.vector.tensor_copy(ob[:], pout[:])
                else:
                    nc.scalar.copy(ob[:], pout[:])
                nc.sync.dma_start(
                    out[128*m:128*m+128, 512*ne:512*ne+512], ob[:])


def _emit_body(nc, tc, a2a_in, a2a_out, tri_sb, pad_sb, ones_sb,
               iden_sb, ht, wqt, wkt, wvt, wot, out,
               do_attn=True, do_a2a=True, do_oproj=True):
    with tc.tile_pool(name="store", bufs=1) as spool:
        qt_sb = [spool.tile([128, NS], MM_DT, name=f"qt{h}") for h in range(HL)]
        kt_sb = [spool.tile([128, NS], MM_DT, name=f"kt{h}") for h in range(HL)]
        v_sb = [spool.tile([128, NS], MM_DT, name=f"v{h}") for h in range(HL)]

        _emit_qkv(nc, tc, qt_sb, kt_sb, v_sb, iden_sb, ht, wqt, wkt, wvt)

        # o_proj pools open before attention so Wo slab DMAs can prefetch
        # into the space vacated by the QKV weight/ht pools during attention.
        with tc.tile_pool(name="oproj", bufs=1) as opool, \
             tc.tile_pool(name="wo", bufs=2) as wopool, \
             tc.tile_pool(name="ob", bufs=3) as obpool:
            if do_attn:
                _emit_attention(nc, tc, qt_sb, kt_sb, v_sb, tri_sb, pad_sb,
                                ones_sb, a2a_in, a2a_out, do_a2a)
            if do_oproj:
                _emit_oproj(nc, tc, opool, wopool, obpool, a2a_out, wot, out)


_NC_CACHE = None


def _get_nc():
    global _NC_CACHE
    if _NC_CACHE is None:
        _NC_CACHE = build_bass()
    return _NC_CACHE


def make_in_maps(hidden_states, attention_mask, Wq, Wk, Wv, Wo):
    import ml_dtypes
    mm_np = np.float32 if MM_DT == F32R else ml_dtypes.bfloat16
    x = np.ascontiguousarray(np.asarray(hidden_states, dtype=np.float32)).reshape(NS, D)
    ht = np.ascontiguousarray(x.T).astype(mm_np)                     # [D, NS]
    wqt = np.ascontiguousarray(np.asarray(Wq, dtype=np.float32).T).astype(mm_np)
    wkt = np.ascontiguousarray(np.asarray(Wk, dtype=np.float32).T).astype(mm_np)
    wvt = np.ascontiguousarray(np.asarray(Wv, dtype=np.float32).T).astype(mm_np)
    import ml_dtypes
    wot = np.ascontiguousarray(
        np.asarray(Wo, dtype=np.float32).T).astype(ml_dtypes.bfloat16)
    mask = np.asarray(attention_mask)
    pad = np.where(mask == 0, np.float32(NEG), np.float32(0.0)).astype(np.float32)
    tri = np.where(
        np.arange(1024, dtype=np.int64)[None, :] >= np.arange(128, dtype=np.int64)[:, None] + 512,
        np.float32(0.0), np.float32(NEG)).astype(np.float32)
    ones = np.ones((128, 128), dtype=np.float32)
    iden = np.eye(128, dtype=np.float32)

    in_maps = []
    for c in range(NC):
        sl = slice(DSL * c, DSL * c + DSL)
        in_maps.append({
            "ht": ht,
            "wqt": np.ascontiguousarray(wqt[:, sl]),
            "wkt": np.ascontiguousarray(wkt[:, sl]),
            "wvt": np.ascontiguousarray(wvt[:, sl]),
            "wot": wot,
            "pad": pad,
            "tri": tri,
            "ones": ones.astype(mm_np),
            "iden": iden.astype(mm_np),
        })
    return in_maps


def assemble_output(results):
    rows = np.concatenate([results[c]["out"] for c in range(NC)], axis=0)
    return rows.reshape(B, S, D).astype(np.float32)


def kernel(hidden_states, attention_mask, Wq, Wk, Wv, Wo):
    nc = _get_nc()
    in_maps = make_in_maps(hidden_states, attention_mask, Wq, Wk, Wv, Wo)
    res = bass_utils.run_bass_kernel_spmd(nc, in_maps, core_ids=list(range(NC)))
    return assemble_output(res.results)

